# revision 39
# baseline (speedup 1.0000x reference)
"""Trainium2 Bass kernel for the binarized CNN (nn_CNN_binary_55001351193058).

Pure data-parallel over 8 NeuronCores (batch-sharded, 1024 samples/core).

v2 design (vs the separate-boundary-matmul baseline):
  - L1: fp16 hi/lo two-pass matmuls (exact to fp32; fp32r single-pass
    flips too many near-threshold binarizations).
  - L2/L3: fp8 DoubleRow matmuls (K=256 in one pass): the conv boundary
    taps ride in the second pair-slot, collapsing main+boundary (L2) and
    the two-pass 192-contraction (L3) into single matmuls.
  - L4: plain bf16 matmuls (strided rhs over u-parity); fc: DoubleRow
    pairs. L4+fc are emitted two chunks late and L3 one chunk late (a
    2-deep software pipeline) so the PE never idles past the HAM window
    and evictions never head-of-line-block the ACT queue.
  - Evictions: maxpool groups are 2 ops total — ACT Sign (side A -> +-1
    bf16) + DVE scalar_tensor_tensor (side B: [z>=t] fused with max-merge
    -> {0,1} fp8). max(+-1,{0,1}) == OR in {0,1} encoding. {0,1} tensors
    feed doubled (+-2) weights with the weight-sum folded into thresholds.
    s1 halo tiles hold 0.5 so the zero-pad decodes exactly to 0.
  - All psum tiles are 2-bank [128,1024] pairs so every eviction reads
    [128,2,384] wide, amortizing the per-op engine overheads.
Exact small-integer arithmetic in layers 2-4 + fc; BatchNorm+Hardtanh+
binarize folded into per-channel thresholds computed on the host in f64.
"""

import numpy as np

import concourse.bass as bass
import concourse.mybir as mybir
import concourse.tile as tile
from concourse import bacc
from concourse.bass_utils import run_bass_kernel_spmd

F32 = mybir.dt.float32
F32R = mybir.dt.float32r
F16 = mybir.dt.float16
BF16 = mybir.dt.bfloat16
FP8 = mybir.dt.float8e4
AF = mybir.ActivationFunctionType
ALU = mybir.AluOpType
DR = mybir.MatmulPerfMode.DoubleRow

B_TOTAL = 8192
N_CORES = 8
B_CORE = B_TOTAL // N_CORES          # 1024
NB = 64                              # samples per chunk
N_CHUNKS = B_CORE // NB              # 16
BH = NB * 6                          # 384 (h,b) columns per chunk
EPS = 1e-5

f8 = mybir.dt.np(FP8)

# maxpool groups whose side-A threshold runs on ACT (rest on DVE):
L1_ACT = (0, 1, 2)          # of 4 groups
L3_ACT = (0, 1, 2, 3, 4, 5)  # of 8 groups


# ----------------------------------------------------------------------------
# Host-side weight preparation (float64 where it matters)
# ----------------------------------------------------------------------------

def _sgn(w):
    return np.where(w >= 0, 1.0, -1.0)


def _threshold(g, be, m, v, bias):
    inv = g.astype(np.float64) / np.sqrt(v.astype(np.float64) + EPS)
    assert (inv > 0).all(), "BN scale must be positive for threshold folding"
    sh = be.astype(np.float64) - m.astype(np.float64) * inv
    return -bias.astype(np.float64) - sh / inv


def _check_margin(th, name, grid=1.0):
    # psum values are exact integers in fp32; the threshold's f32 rounding
    # error is ~1e-5*|th|, so any margin comfortably above that is safe.
    d = np.abs(th / grid - np.round(th / grid)) * grid
    if d.min() < 1e-4:
        raise AssertionError(f"threshold margin too small for {name}: {d.min()}")


def prepare_host_tensors(w1, b1, w2, b2, w3, b3, w4, b4,
                         g1, be1, m1, v1, g2, be2, m2, v2,
                         g3, be3, m3, v3, g4, be4, m4, v4, wf, bf):
    t1 = _threshold(g1, be1, m1, v1, b1)       # [32]
    t2 = _threshold(g2, be2, m2, v2, b2)       # [64]
    t3 = _threshold(g3, be3, m3, v3, b3)       # [128]
    t4 = _threshold(g4, be4, m4, v4, b4)       # [128]

    s1w = _sgn(w1)[:, 0, 0, :]                 # [32, 9]
    s2 = _sgn(w2)[:, :, 0, :]                  # [64, 32, 3]
    s3w = _sgn(w3)[:, :, 0, :]                 # [128, 64, 3]
    s4w = _sgn(w4)[:, :, :, 0]                 # [128, 128, 6]
    sf = _sgn(wf)                              # [10, 2048]

    # decode-compensation constants for {0,1}-encoded inputs (weights x2)
    c2 = s2.sum(axis=(1, 2))                   # [64]
    c4 = s4w.sum(axis=(1, 2))                  # [128]

    # psums land on the even-integer lattice: L2 = 2*sum(w*g) + (even # of
    # +-1 halo terms, ci=32); L3 = even # of +-1 terms; L4 = 2*sum(w*g).
    _check_margin(t2 + c2, "t2+c2", grid=2.0)
    _check_margin(t3, "t3", grid=2.0)
    _check_margin(t4 + c4, "t4+c4", grid=2.0)

    # L1: 16 m-tiles (8 u x even/odd), lhsT [w, (p,ci)], fp32.
    # row (p,ci) of tile m=2u+half holds conv1 out at wy = 2*(4u+p)+half:
    #   wx = 2*wy + k - 4
    A1 = np.zeros((16, 128, 128), np.float32)
    for u in range(8):
        for half in range(2):
            m = 2 * u + half
            for p in range(4):
                wy = 2 * (4 * u + p) + half
                for k in range(9):
                    wx = 2 * wy + k - 4
                    if 0 <= wx < 128:
                        A1[m, wx, p * 32:(p + 1) * 32] = s1w[:, k]
    A1f = A1.transpose(1, 0, 2).reshape(128, 16 * 128)
    # Row-tiled layout: pair p covers u0=2p (strip0, rows 0:64) and u1=2p+1
    # (strip1, rows 64:128); strip r holds x rows [16*u-4, 16*u+60).
    # col block (p, r, half) at p*512 + r*256 + half*128.
    A1R = np.zeros((128, 2048), np.float32)
    for pp in range(4):
        for r in range(2):
            u = 2 * pp + r
            base = 16 * u - 4
            for rr in range(64):
                w = base + rr
                if 0 <= w < 128:
                    for half in range(2):
                        m = 2 * u + half
                        A1R[64 * r + rr,
                            pp * 512 + r * 256 + half * 128:
                            pp * 512 + r * 256 + half * 128 + 128] = \
                            A1f[w, m * 128:(m + 1) * 128]
    # equivalence check: strip matmul == full matmul on random data
    rng = np.random.default_rng(1)
    xt = rng.standard_normal((128, 8)).astype(np.float32)
    for pp in range(4):
        for r in range(2):
            u = 2 * pp + r
            base = 16 * u - 4
            xs = np.zeros((64, 8), np.float32)
            for rr in range(64):
                if 0 <= base + rr < 128:
                    xs[rr] = xt[base + rr]
            for half in range(2):
                m = 2 * u + half
                got = A1R[64 * r:64 * r + 64,
                          pp * 512 + r * 256 + half * 128:
                          pp * 512 + r * 256 + half * 128 + 128].T @ xs
                want = A1f[:, m * 128:(m + 1) * 128].T @ xt
                assert np.abs(got - want).max() < 1e-4, (pp, r, half)

    # L2 stationaries [(p,ci), (op,co)], weights doubled (s1 is {0,1}).
    # even v (out pos 4u+op):  k = p - op + 1
    # odd  v (out pos 4u+2+op): k = p - op - 1
    W2e = np.zeros((128, 128), np.float64)
    W2o = np.zeros((128, 128), np.float64)
    for p in range(4):
        for op in range(2):
            ke = p - op + 1
            if 0 <= ke <= 2:
                W2e[p * 32:(p + 1) * 32, op * 64:(op + 1) * 64] = s2[:, :, ke].T
            ko = p - op - 1
            if 0 <= ko <= 2:
                W2o[p * 32:(p + 1) * 32, op * 64:(op + 1) * 64] = s2[:, :, ko].T
    # boundary taps: even v op0 k0 from prev tile p3; odd v op1 k2 from next p0
    W2eb = np.zeros((128, 128), np.float64)
    W2eb[96:128, 0:64] = s2[:, :, 0].T
    W2ob = np.zeros((128, 128), np.float64)
    W2ob[0:32, 64:128] = s2[:, :, 2].T
    # DoubleRow pair-stationaries: slot0 = first rhs tile, slot1 = second.
    Se = np.concatenate([2 * W2eb, 2 * W2e], axis=1)     # [128, 256]
    So = np.concatenate([2 * W2o, 2 * W2ob], axis=1)

    # L3 stationaries, q rows (op, co2), +-1 (q is +-1).
    W3aL = np.zeros((128, 128), np.float64)   # mid taps, pos-left (2u)
    W3aR = np.zeros((128, 128), np.float64)   # mid taps, pos-right (2u+1)
    W3aL[0:64, :] = s3w[:, :, 1].T
    W3aL[64:128, :] = s3w[:, :, 2].T
    W3aR[0:64, :] = s3w[:, :, 0].T
    W3aR[64:128, :] = s3w[:, :, 1].T
    W3bL = np.zeros((128, 128), np.float64)   # prev-tile taps for pos-left
    W3bL[64:128, :] = s3w[:, :, 0].T
    W3bR = np.zeros((128, 128), np.float64)   # next-tile taps for pos-right
    W3bR[0:64, :] = s3w[:, :, 2].T
    Sa = np.concatenate([W3bL, W3aL], axis=1)            # [128, 256]
    Sb = np.concatenate([W3aR, W3bR], axis=1)

    # L4 [ci, (h,co)], doubled (s3 is {0,1})
    W4t = 2 * s4w.transpose(2, 1, 0).reshape(6, 128, 128)
    W4t = W4t.transpose(1, 0, 2).reshape(128, 6 * 128)

    # fc DoubleRow pairs: pair k = (w=2k, w=2k+1), 32-col stride, 10 used
    Wf3 = sf.reshape(10, 128, 16)                        # [j, co, w]
    Wfp = np.zeros((128, 8 * 32), np.float64)
    for k in range(8):
        Wfp[:, 32 * k:32 * k + 10] = Wf3[:, :, 2 * k].T
        Wfp[:, 32 * k + 16:32 * k + 26] = Wf3[:, :, 2 * k + 1].T

    t1v = np.tile(t1, 4).reshape(128, 1)
    b2v = -(np.concatenate([t2, t2]) + np.concatenate([c2, c2])).reshape(128, 1)
    t3v = t3.reshape(128, 1)
    b4v = -(t4 + c4).reshape(128, 1)

    return dict(
        A1=A1R.astype(np.float16),
        Se=Se.astype(f8), So=So.astype(f8),
        Sa=Sa.astype(f8), Sb=Sb.astype(f8),
        W4t=W4t.astype(mybir.dt.np(BF16)), Wfp=Wfp.astype(f8),
        t1v=t1v.astype(np.float32), nt1v=(-t1v).astype(np.float32),
        b2v=b2v.astype(np.float32),
        t3v=t3v.astype(np.float32), nt3v=(-t3v).astype(np.float32),
        b4v=b4v.astype(np.float32),
        bfv=bf.astype(np.float32).reshape(10, 1),
    )


# ----------------------------------------------------------------------------
# Bass program (identical SPMD program for each core)
# ----------------------------------------------------------------------------

def build_program():
    nc = bacc.Bacc("TRN2", target_bir_lowering=False, debug=False)

    xh_d = nc.dram_tensor("xh", [128, 4 * B_CORE * 6], F16, kind="ExternalInput").ap()
    xl_d = nc.dram_tensor("xl", [128, 4 * B_CORE * 6], F16, kind="ExternalInput").ap()
    A1_d = nc.dram_tensor("A1", [128, 2048], F16, kind="ExternalInput").ap()
    Se_d = nc.dram_tensor("Se", [128, 256], FP8, kind="ExternalInput").ap()
    So_d = nc.dram_tensor("So", [128, 256], FP8, kind="ExternalInput").ap()
    Sa_d = nc.dram_tensor("Sa", [128, 256], FP8, kind="ExternalInput").ap()
    Sb_d = nc.dram_tensor("Sb", [128, 256], FP8, kind="ExternalInput").ap()
    W4_d = nc.dram_tensor("W4t", [128, 6 * 128], BF16, kind="ExternalInput").ap()
    Wf_d = nc.dram_tensor("Wfp", [128, 256], FP8, kind="ExternalInput").ap()
    t1_d = nc.dram_tensor("t1v", [128, 1], F32, kind="ExternalInput").ap()
    nt1_d = nc.dram_tensor("nt1v", [128, 1], F32, kind="ExternalInput").ap()
    b2_d = nc.dram_tensor("b2v", [128, 1], F32, kind="ExternalInput").ap()
    t3_d = nc.dram_tensor("t3v", [128, 1], F32, kind="ExternalInput").ap()
    nt3_d = nc.dram_tensor("nt3v", [128, 1], F32, kind="ExternalInput").ap()
    b4_d = nc.dram_tensor("b4v", [128, 1], F32, kind="ExternalInput").ap()
    bf_d = nc.dram_tensor("bfv", [10, 1], F32, kind="ExternalInput").ap()

    y_d = nc.dram_tensor("y", [B_CORE, 10], F32, kind="ExternalOutput").ap()

    with tile.TileContext(nc) as tc:
        with (
            tc.tile_pool(name="consts", bufs=1) as consts,
            tc.tile_pool(name="xin", bufs=18) as xin_pool,
            tc.tile_pool(name="fbuf", bufs=6) as f_pool,
            tc.tile_pool(name="s1buf", bufs=2) as s1_pool,
            tc.tile_pool(name="qbuf", bufs=2) as q_pool,
            tc.tile_pool(name="s3buf", bufs=3) as s3_pool,
            tc.tile_pool(name="s4buf", bufs=2) as s4_pool,
            tc.tile_pool(name="oc", bufs=3) as oc_pool,
            tc.tile_pool(name="psM", bufs=4, space="PSUM") as psM_pool,
        ):
            # --- constants ---
            # Tiles allocated in the original order; only the dma_start calls
            # for non-critical consts are deferred until after chunk-0's x
            # DMAs, so the first L1 matmuls start ~10 us earlier.
            deferred = []
            A1_s = consts.tile([128, 2048], F16)
            nc.sync.dma_start(out=A1_s, in_=A1_d)
            Se_s = consts.tile([128, 256], FP8)
            deferred.append((Se_s, Se_d))
            So_s = consts.tile([128, 256], FP8)
            deferred.append((So_s, So_d))
            Sa_s = consts.tile([128, 256], FP8)
            deferred.append((Sa_s, Sa_d))
            Sb_s = consts.tile([128, 256], FP8)
            deferred.append((Sb_s, Sb_d))
            W4_s = consts.tile([128, 6 * 128], BF16)
            deferred.append((W4_s, W4_d))
            Wf_s = consts.tile([128, 256], FP8)
            deferred.append((Wf_s, Wf_d))
            t1_s = consts.tile([128, 1], F32)
            nt1_s = consts.tile([128, 1], F32)
            b2_s = consts.tile([128, 1], F32)
            deferred.append((b2_s, b2_d))
            t3_s = consts.tile([128, 1], F32)
            deferred.append((t3_s, t3_d))
            nt3_s = consts.tile([128, 1], F32)
            deferred.append((nt3_s, nt3_d))
            b4_s = consts.tile([128, 1], F32)
            deferred.append((b4_s, b4_d))
            bf_s = consts.tile([10, 1], F32)
            deferred.append((bf_s, bf_d))

            pair2 = dict(two=2)

            def p2(ap):
                return ap.rearrange("p (two n) -> p two n", **pair2)

            # persistent double buffers (halos set once)
            s1_bufs, q_bufs, s3_bufs, s4_bufs = [], [], [], []
            for i in range(2):
                s1b = s1_pool.tile([128, 10 * BH], FP8, name=f"s1b{i}")
                nc.vector.memset(s1b[:, 0:BH], 0.5)          # pad decodes to 0
                nc.vector.memset(s1b[:, 9 * BH:10 * BH], 0.5)
                s1_bufs.append(s1b)
                qb = q_pool.tile([128, 18 * BH], FP8, name=f"qb{i}")
                nc.vector.memset(qb[:, 0:BH], 0.0)
                nc.vector.memset(qb[:, 17 * BH:18 * BH], 0.0)
                q_bufs.append(qb)
                s3_bufs.append(s3_pool.tile([128, 16 * BH], BF16, name=f"s3b{i}"))
                s4_bufs.append(s4_pool.tile([128, 1024], FP8, name=f"s4b{i}"))
            s3_bufs.append(s3_pool.tile([128, 16 * BH], BF16, name="s3b2"))

            def issue_x(c):
                tiles = []
                for pp in range(4):
                    off = pp * B_CORE * 6 + c * BH
                    xh = xin_pool.tile([128, BH], F16, tag="xh",
                                       name=f"xh_{c}_{pp}")
                    nc.sync.dma_start(out=xh, in_=xh_d[:, off:off + BH])
                    xl = xin_pool.tile([128, BH], F16, tag="xl",
                                       name=f"xl_{c}_{pp}")
                    nc.sync.dma_start(out=xl, in_=xl_d[:, off:off + BH])
                    tiles.append((xh, xl))
                return tiles

            def emit_l4(c):
                # L4: plain bf16, contract (ci,h) over u-parity halves
                s3c = s3_bufs[c % 3]
                s4c = s4_bufs[c % 2]
                s3v = s3c.rearrange("p (u h b) -> p u h b", h=6, b=NB)
                ps4 = psM_pool.tile([128, 1024], F32, tag="psM")
                for h in range(6):
                    for par in range(2):
                        nc.tensor.matmul(
                            ps4[:, 512 * par:512 * (par + 1)],
                            W4_s[:, 128 * h:128 * (h + 1)],
                            s3v[:, par:16:2, h, :],
                            start=(h == 0), stop=(h == 5))
                nc.scalar.activation(s4c, p2(ps4), AF.Sign, bias=b4_s)

            def emit_fc(c):
                # fc: DoubleRow pairs (w=2k, w=2k+1); s4 evicted a full
                # pipeline phase ago, so these matmuls never wait
                s4c = s4_bufs[c % 2]
                psf = psM_pool.tile([10, 64], F32, tag="psM")
                s4v = p2(s4c)
                for k in range(8):
                    nc.tensor.matmul(
                        psf,
                        p2(Wf_s[:, 32 * k:32 * k + 32])[:, :, 0:10],
                        s4v[:, :, 64 * k:64 * (k + 1)],
                        start=(k == 0), stop=(k == 7), perf_mode=DR)
                outc = oc_pool.tile([10, NB], F32)
                nc.vector.tensor_scalar_add(outc, psf, bf_s)
                nc.gpsimd.dma_start(
                    out=y_d[c * NB:(c + 1) * NB, :].rearrange("b j -> j b"),
                    in_=outc)

            x_cur = issue_x(0)
            nc.sync.dma_start(out=t1_s, in_=t1_d)
            nc.sync.dma_start(out=nt1_s, in_=nt1_d)
            for t, d in deferred:
                nc.sync.dma_start(out=t, in_=d)

            def pool_evict(psA, psB, dest, t_s, nt_s, on_act):
                # dest <- OR([psA>=t],[psB>=t]) in {0,1} fp8 (wide pair)
                fa = f_pool.tile([128, 2 * BH], BF16, tag="fa")
                if on_act:
                    nc.scalar.activation(fa, p2(psA)[:, :, 0:BH],
                                         AF.Sign, bias=nt_s)
                else:
                    nc.vector.tensor_scalar(
                        out=fa, in0=p2(psA)[:, :, 0:BH],
                        scalar1=t_s, scalar2=None, op0=ALU.is_ge)
                nc.vector.scalar_tensor_tensor(
                    dest, p2(psB)[:, :, 0:BH], t_s, fa,
                    op0=ALU.is_ge, op1=ALU.max)

            def emit_l1(c, xtiles):
                s1t = s1_bufs[c % 2]
                for g in range(4):                # pair g: strips r=0,1
                    xh, xl = xtiles[g]
                    psE = psM_pool.tile([128, 1024], F32, tag="psM",
                                        name=f"psE_{c}_{g}")
                    psO = psM_pool.tile([128, 1024], F32, tag="psM",
                                        name=f"psO_{c}_{g}")
                    for half, ps in ((0, psE), (1, psO)):
                        for r in range(2):        # two strips run concurrent
                            a = A1_s[64 * r:64 * r + 64,
                                     g * 512 + r * 256 + half * 128:
                                     g * 512 + r * 256 + half * 128 + 128]
                            nc.tensor.matmul(
                                ps[:, 512 * r:512 * r + BH], a,
                                xh[64 * r:64 * r + 64, :],
                                start=True, stop=False,
                                tile_position=(64 * r, 0))
                        for r in range(2):
                            a = A1_s[64 * r:64 * r + 64,
                                     g * 512 + r * 256 + half * 128:
                                     g * 512 + r * 256 + half * 128 + 128]
                            nc.tensor.matmul(
                                ps[:, 512 * r:512 * r + BH], a,
                                xl[64 * r:64 * r + 64, :],
                                start=False, stop=True,
                                tile_position=(64 * r, 0))
                    pool_evict(psE, psO,
                               s1t[:, (2 * g + 1) * BH:(2 * g + 3) * BH],
                               t1_s, nt1_s, g in L1_ACT)

            def emit_l2(c):
                s1t = s1_bufs[c % 2]
                qt = q_bufs[c % 2]
                for g in range(2):
                    pss = [psM_pool.tile([128, 1024], F32, tag="psM",
                                         name=f"ps2_{c}_{g}_{i}")
                           for i in range(4)]

                    def mm_e(i):                          # even-v MM (Se)
                        u = 4 * g + i
                        nc.tensor.matmul(
                            pss[i][:, 0:BH], p2(Se_s),
                            p2(s1t[:, u * BH:(u + 2) * BH]),
                            start=True, stop=True, perf_mode=DR)

                    def mm_o(i):                          # odd-v MM (So)
                        u = 4 * g + i
                        nc.tensor.matmul(
                            pss[i][:, 512:512 + BH], p2(So_s),
                            p2(s1t[:, (u + 1) * BH:(u + 3) * BH]),
                            start=True, stop=True, perf_mode=DR)

                    first, second = (mm_e, mm_o) if g % 2 == 0 else (mm_o, mm_e)
                    for i in range(4):
                        first(i)
                    for i in range(4):
                        second(i)
                    for i, ps in enumerate(pss):          # q tiles v=2u, 2u+1
                        u = 4 * g + i
                        nc.scalar.activation(
                            qt[:, (2 * u + 1) * BH:(2 * u + 3) * BH],
                            p2(ps)[:, :, 0:BH], AF.Sign, bias=b2_s)

            def emit_l3(c):
                qt = q_bufs[c % 2]
                s3t = s3_bufs[c % 3]
                for g in range(8):
                    psA = psM_pool.tile([128, 1024], F32, tag="psM",
                                        name=f"psA_{c}_{g}")
                    psB = psM_pool.tile([128, 1024], F32, tag="psM",
                                        name=f"psB_{c}_{g}")

                    def mm_a(i):                  # pos-left MM (Sa)
                        u3 = 2 * g + i
                        nc.tensor.matmul(
                            psA[:, 512 * i:512 * i + BH], p2(Sa_s),
                            p2(qt[:, u3 * BH:(u3 + 2) * BH]),
                            start=True, stop=True, perf_mode=DR)

                    def mm_b(i):                  # pos-right MM (Sb)
                        u3 = 2 * g + i
                        nc.tensor.matmul(
                            psB[:, 512 * i:512 * i + BH], p2(Sb_s),
                            p2(qt[:, (u3 + 1) * BH:(u3 + 3) * BH]),
                            start=True, stop=True, perf_mode=DR)

                    # alternate stationary order per group: runs of 4 MMs
                    # share one LDWEIGHTS across the group boundary
                    first, second = (mm_a, mm_b) if g % 2 == 0 else (mm_b, mm_a)
                    first(0), first(1), second(0), second(1)
                    # evict the earlier-finished side first (OR is symmetric)
                    pa, pb = (psA, psB) if g % 2 == 0 else (psB, psA)
                    pool_evict(pa, pb,
                               s3t[:, (2 * g) * BH:(2 * g + 2) * BH],
                               t3_s, nt3_s, g in L3_ACT)

            # 2-deep software pipeline: iteration it emits
            #   L3(it-1) | L1(it) | L2(it) | L4+fc(it-2)
            # (L3 first: its q inputs resolved a full iteration ago, so the
            # PE starts immediately instead of waiting on the psum rotation)
            # fully-decoupled lag chain: every inter-layer edge is
            # cross-iteration (L1 it | L2 it-1 | L3 it-2 | L4 it-4 | fc it-5)
            for it in range(N_CHUNKS + 5):
                if 2 <= it <= N_CHUNKS + 1:
                    emit_l3(it - 2)
                if it >= 5:
                    emit_fc(it - 5)
                if it < N_CHUNKS:
                    xtiles = x_cur
                    if it + 1 < N_CHUNKS:
                        x_cur = issue_x(it + 1)
                    emit_l1(it, xtiles)
                if 1 <= it <= N_CHUNKS:
                    emit_l2(it - 1)
                if 4 <= it <= N_CHUNKS + 3:
                    emit_l4(it - 4)

    nc.compile()
    return nc


_PROGRAM = None


def _get_program():
    global _PROGRAM
    if _PROGRAM is None:
        _PROGRAM = build_program()
    return _PROGRAM


def run(trace=False, **inputs):
    inputs = {k: np.asarray(v) for k, v in inputs.items()}
    consts = prepare_host_tensors(
        **{k: inputs[k] for k in
           ("w1", "b1", "w2", "b2", "w3", "b3", "w4", "b4",
            "g1", "be1", "m1", "v1", "g2", "be2", "m2", "v2",
            "g3", "be3", "m3", "v3", "g4", "be4", "m4", "v4", "wf", "bf")})
    x = inputs["x"].astype(np.float32)           # [8192, 1, 6, 128]
    nc = _get_program()
    in_maps = []
    for k in range(N_CORES):
        xc = x[k * B_CORE:(k + 1) * B_CORE, 0]               # [1024, 6, 128]
        # chunk-column order (c, h, b) so L4's rhs slices are contiguous
        xT = np.ascontiguousarray(
            xc.reshape(N_CHUNKS, NB, 6, 128).transpose(0, 2, 1, 3)
            .reshape(B_CORE * 6, 128).T)
        # row-remapped pair layouts: pair p strip r <- x rows 16*(2p+r)-4 ..
        xq = np.zeros((128, 4 * B_CORE * 6), np.float32)
        for pp in range(4):
            for r in range(2):
                base = 16 * (2 * pp + r) - 4
                lo, hi = max(0, -base), min(64, 128 - base)
                xq[64 * r + lo:64 * r + hi,
                   pp * B_CORE * 6:(pp + 1) * B_CORE * 6] = \
                    xT[base + lo:base + hi, :]
        xh = xq.astype(np.float16)
        xl = (xq - xh.astype(np.float32)).astype(np.float16)
        m = {"xh": xh, "xl": xl}
        m.update(consts)
        in_maps.append(m)
    res = run_bass_kernel_spmd(nc, in_maps, list(range(N_CORES)), trace=trace)
    y = np.concatenate([r["y"] for r in res.results], axis=0)
    return y.astype(np.float32), res


def kernel(**inputs):
    y, _ = run(trace=False, **inputs)
    return y


# revision 40
# speedup vs baseline: 1.2149x; 1.2149x over previous
"""Trainium2 Bass kernel for the binarized CNN (nn_CNN_binary_55001351193058).

Pure data-parallel over 8 NeuronCores (batch-sharded, 1024 samples/core).

v2 design (vs the separate-boundary-matmul baseline):
  - L1: fp16 hi/lo two-pass matmuls (exact to fp32; fp32r single-pass
    flips too many near-threshold binarizations).
  - L2/L3: fp8 DoubleRow matmuls (K=256 in one pass): the conv boundary
    taps ride in the second pair-slot, collapsing main+boundary (L2) and
    the two-pass 192-contraction (L3) into single matmuls.
  - L4: plain bf16 matmuls (strided rhs over u-parity); fc: DoubleRow
    pairs. L4+fc are emitted two chunks late and L3 one chunk late (a
    2-deep software pipeline) so the PE never idles past the HAM window
    and evictions never head-of-line-block the ACT queue.
  - Evictions: maxpool groups are 2 ops total — ACT Sign (side A -> +-1
    bf16) + DVE scalar_tensor_tensor (side B: [z>=t] fused with max-merge
    -> {0,1} fp8). max(+-1,{0,1}) == OR in {0,1} encoding. {0,1} tensors
    feed doubled (+-2) weights with the weight-sum folded into thresholds.
    s1 halo tiles hold 0.5 so the zero-pad decodes exactly to 0.
  - All psum tiles are 2-bank [128,1024] pairs so every eviction reads
    [128,2,384] wide, amortizing the per-op engine overheads.
Exact small-integer arithmetic in layers 2-4 + fc; BatchNorm+Hardtanh+
binarize folded into per-channel thresholds computed on the host in f64.
"""

import numpy as np

import concourse.bass as bass
import concourse.mybir as mybir
import concourse.tile as tile
from concourse import bacc
from concourse.bass_utils import run_bass_kernel_spmd

F32 = mybir.dt.float32
F32R = mybir.dt.float32r
F16 = mybir.dt.float16
BF16 = mybir.dt.bfloat16
FP8 = mybir.dt.float8e4
AF = mybir.ActivationFunctionType
ALU = mybir.AluOpType
DR = mybir.MatmulPerfMode.DoubleRow

B_TOTAL = 8192
N_CORES = 8
B_CORE = B_TOTAL // N_CORES          # 1024
NB = 64                              # samples per chunk
N_CHUNKS = B_CORE // NB              # 16
BH = NB * 6                          # 384 (h,b) columns per chunk
EPS = 1e-5

f8 = mybir.dt.np(FP8)

# maxpool groups whose side-A threshold runs on ACT (rest on DVE):
L1_ACT = (0, 1, 2)          # of 4 groups
L3_ACT = (0, 1, 2, 3, 4, 5)  # of 8 groups


# ----------------------------------------------------------------------------
# Host-side weight preparation (float64 where it matters)
# ----------------------------------------------------------------------------

def _sgn(w):
    return np.where(w >= 0, 1.0, -1.0)


def _threshold(g, be, m, v, bias):
    inv = g.astype(np.float64) / np.sqrt(v.astype(np.float64) + EPS)
    assert (inv > 0).all(), "BN scale must be positive for threshold folding"
    sh = be.astype(np.float64) - m.astype(np.float64) * inv
    return -bias.astype(np.float64) - sh / inv


def _check_margin(th, name, grid=1.0):
    # psum values are exact integers in fp32; the threshold's f32 rounding
    # error is ~1e-5*|th|, so any margin comfortably above that is safe.
    d = np.abs(th / grid - np.round(th / grid)) * grid
    if d.min() < 1e-4:
        raise AssertionError(f"threshold margin too small for {name}: {d.min()}")


def prepare_host_tensors(w1, b1, w2, b2, w3, b3, w4, b4,
                         g1, be1, m1, v1, g2, be2, m2, v2,
                         g3, be3, m3, v3, g4, be4, m4, v4, wf, bf):
    t1 = _threshold(g1, be1, m1, v1, b1)       # [32]
    t2 = _threshold(g2, be2, m2, v2, b2)       # [64]
    t3 = _threshold(g3, be3, m3, v3, b3)       # [128]
    t4 = _threshold(g4, be4, m4, v4, b4)       # [128]

    s1w = _sgn(w1)[:, 0, 0, :]                 # [32, 9]
    s2 = _sgn(w2)[:, :, 0, :]                  # [64, 32, 3]
    s3w = _sgn(w3)[:, :, 0, :]                 # [128, 64, 3]
    s4w = _sgn(w4)[:, :, :, 0]                 # [128, 128, 6]
    sf = _sgn(wf)                              # [10, 2048]

    # decode-compensation constants for {0,1}-encoded inputs (weights x2)
    c2 = s2.sum(axis=(1, 2))                   # [64]
    c4 = s4w.sum(axis=(1, 2))                  # [128]

    # psums land on the even-integer lattice: L2 = 2*sum(w*g) + (even # of
    # +-1 halo terms, ci=32); L3 = even # of +-1 terms; L4 = 2*sum(w*g).
    _check_margin(t2 + c2, "t2+c2", grid=2.0)
    _check_margin(t3, "t3", grid=2.0)
    _check_margin(t4 + c4, "t4+c4", grid=2.0)

    # L1: 16 m-tiles (8 u x even/odd), lhsT [w, (p,ci)], fp32.
    # row (p,ci) of tile m=2u+half holds conv1 out at wy = 2*(4u+p)+half:
    #   wx = 2*wy + k - 4
    A1 = np.zeros((16, 128, 128), np.float32)
    for u in range(8):
        for half in range(2):
            m = 2 * u + half
            for p in range(4):
                wy = 2 * (4 * u + p) + half
                for k in range(9):
                    wx = 2 * wy + k - 4
                    if 0 <= wx < 128:
                        A1[m, wx, p * 32:(p + 1) * 32] = s1w[:, k]
    A1f = A1.transpose(1, 0, 2).reshape(128, 16 * 128)
    # Row-tiled layout: pair p covers u0=2p (strip0, rows 0:64) and u1=2p+1
    # (strip1, rows 64:128); strip r holds x rows [16*u-4, 16*u+60).
    # col block (p, r, half) at p*512 + r*256 + half*128.
    A1R = np.zeros((128, 2048), np.float32)
    for pp in range(4):
        for r in range(2):
            u = 2 * pp + r
            base = 16 * u - 4
            for rr in range(64):
                w = base + rr
                if 0 <= w < 128:
                    for half in range(2):
                        m = 2 * u + half
                        A1R[64 * r + rr,
                            pp * 512 + r * 256 + half * 128:
                            pp * 512 + r * 256 + half * 128 + 128] = \
                            A1f[w, m * 128:(m + 1) * 128]
    # equivalence check: strip matmul == full matmul on random data
    rng = np.random.default_rng(1)
    xt = rng.standard_normal((128, 8)).astype(np.float32)
    for pp in range(4):
        for r in range(2):
            u = 2 * pp + r
            base = 16 * u - 4
            xs = np.zeros((64, 8), np.float32)
            for rr in range(64):
                if 0 <= base + rr < 128:
                    xs[rr] = xt[base + rr]
            for half in range(2):
                m = 2 * u + half
                got = A1R[64 * r:64 * r + 64,
                          pp * 512 + r * 256 + half * 128:
                          pp * 512 + r * 256 + half * 128 + 128].T @ xs
                want = A1f[:, m * 128:(m + 1) * 128].T @ xt
                assert np.abs(got - want).max() < 1e-4, (pp, r, half)

    # L2 stationaries [(p,ci), (op,co)], weights doubled (s1 is {0,1}).
    # even v (out pos 4u+op):  k = p - op + 1
    # odd  v (out pos 4u+2+op): k = p - op - 1
    W2e = np.zeros((128, 128), np.float64)
    W2o = np.zeros((128, 128), np.float64)
    for p in range(4):
        for op in range(2):
            ke = p - op + 1
            if 0 <= ke <= 2:
                W2e[p * 32:(p + 1) * 32, op * 64:(op + 1) * 64] = s2[:, :, ke].T
            ko = p - op - 1
            if 0 <= ko <= 2:
                W2o[p * 32:(p + 1) * 32, op * 64:(op + 1) * 64] = s2[:, :, ko].T
    # boundary taps: even v op0 k0 from prev tile p3; odd v op1 k2 from next p0
    W2eb = np.zeros((128, 128), np.float64)
    W2eb[96:128, 0:64] = s2[:, :, 0].T
    W2ob = np.zeros((128, 128), np.float64)
    W2ob[0:32, 64:128] = s2[:, :, 2].T
    # DoubleRow pair-stationaries: slot0 = first rhs tile, slot1 = second.
    Se = np.concatenate([2 * W2eb, 2 * W2e], axis=1)     # [128, 256]
    So = np.concatenate([2 * W2o, 2 * W2ob], axis=1)

    # L3 stationaries, q rows (op, co2), +-1 (q is +-1).
    W3aL = np.zeros((128, 128), np.float64)   # mid taps, pos-left (2u)
    W3aR = np.zeros((128, 128), np.float64)   # mid taps, pos-right (2u+1)
    W3aL[0:64, :] = s3w[:, :, 1].T
    W3aL[64:128, :] = s3w[:, :, 2].T
    W3aR[0:64, :] = s3w[:, :, 0].T
    W3aR[64:128, :] = s3w[:, :, 1].T
    W3bL = np.zeros((128, 128), np.float64)   # prev-tile taps for pos-left
    W3bL[64:128, :] = s3w[:, :, 0].T
    W3bR = np.zeros((128, 128), np.float64)   # next-tile taps for pos-right
    W3bR[0:64, :] = s3w[:, :, 2].T
    Sa = np.concatenate([W3bL, W3aL], axis=1)            # [128, 256]
    Sb = np.concatenate([W3aR, W3bR], axis=1)

    # L4 [ci, (h,co)], doubled (s3 is {0,1})
    W4t = 2 * s4w.transpose(2, 1, 0).reshape(6, 128, 128)
    W4t = W4t.transpose(1, 0, 2).reshape(128, 6 * 128)

    # fc DoubleRow pairs: pair k = (w=2k, w=2k+1), 32-col stride, 10 used
    Wf3 = sf.reshape(10, 128, 16)                        # [j, co, w]
    Wfp = np.zeros((128, 8 * 32), np.float64)
    for k in range(8):
        Wfp[:, 32 * k:32 * k + 10] = Wf3[:, :, 2 * k].T
        Wfp[:, 32 * k + 16:32 * k + 26] = Wf3[:, :, 2 * k + 1].T

    t1v = np.tile(t1, 4).reshape(128, 1)
    b2v = -(np.concatenate([t2, t2]) + np.concatenate([c2, c2])).reshape(128, 1)
    t3v = t3.reshape(128, 1)
    b4v = -(t4 + c4).reshape(128, 1)

    return dict(
        A1=A1R.astype(np.float16),
        Se=Se.astype(f8), So=So.astype(f8),
        Sa=Sa.astype(f8), Sb=Sb.astype(f8),
        W4t=W4t.astype(mybir.dt.np(BF16)), Wfp=Wfp.astype(f8),
        t1v=t1v.astype(np.float32), nt1v=(-t1v).astype(np.float32),
        b2v=b2v.astype(np.float32),
        t3v=t3v.astype(np.float32), nt3v=(-t3v).astype(np.float32),
        b4v=b4v.astype(np.float32),
        bfv=bf.astype(np.float32).reshape(10, 1),
    )


# ----------------------------------------------------------------------------
# Bass program (identical SPMD program for each core)
# ----------------------------------------------------------------------------

def build_program():
    nc = bacc.Bacc("TRN2", target_bir_lowering=False, debug=False)

    xh_d = nc.dram_tensor("xh", [128, 4 * B_CORE * 6], F16, kind="ExternalInput").ap()
    xl_d = nc.dram_tensor("xl", [128, 4 * B_CORE * 6], F16, kind="ExternalInput").ap()
    A1_d = nc.dram_tensor("A1", [128, 2048], F16, kind="ExternalInput").ap()
    Se_d = nc.dram_tensor("Se", [128, 256], FP8, kind="ExternalInput").ap()
    So_d = nc.dram_tensor("So", [128, 256], FP8, kind="ExternalInput").ap()
    Sa_d = nc.dram_tensor("Sa", [128, 256], FP8, kind="ExternalInput").ap()
    Sb_d = nc.dram_tensor("Sb", [128, 256], FP8, kind="ExternalInput").ap()
    W4_d = nc.dram_tensor("W4t", [128, 6 * 128], BF16, kind="ExternalInput").ap()
    Wf_d = nc.dram_tensor("Wfp", [128, 256], FP8, kind="ExternalInput").ap()
    t1_d = nc.dram_tensor("t1v", [128, 1], F32, kind="ExternalInput").ap()
    nt1_d = nc.dram_tensor("nt1v", [128, 1], F32, kind="ExternalInput").ap()
    b2_d = nc.dram_tensor("b2v", [128, 1], F32, kind="ExternalInput").ap()
    t3_d = nc.dram_tensor("t3v", [128, 1], F32, kind="ExternalInput").ap()
    nt3_d = nc.dram_tensor("nt3v", [128, 1], F32, kind="ExternalInput").ap()
    b4_d = nc.dram_tensor("b4v", [128, 1], F32, kind="ExternalInput").ap()
    bf_d = nc.dram_tensor("bfv", [10, 1], F32, kind="ExternalInput").ap()

    y_d = nc.dram_tensor("y", [B_CORE, 10], F32, kind="ExternalOutput").ap()

    with tile.TileContext(nc) as tc:
        with (
            tc.tile_pool(name="consts", bufs=1) as consts,
            tc.tile_pool(name="xin", bufs=18) as xin_pool,
            tc.tile_pool(name="fbuf", bufs=6) as f_pool,
            tc.tile_pool(name="s1buf", bufs=2) as s1_pool,
            tc.tile_pool(name="qbuf", bufs=2) as q_pool,
            tc.tile_pool(name="s3buf", bufs=2) as s3_pool,
            tc.tile_pool(name="s4buf", bufs=2) as s4_pool,
            tc.tile_pool(name="oc", bufs=3) as oc_pool,
            tc.tile_pool(name="psM", bufs=4, space="PSUM") as psM_pool,
        ):
            # --- constants ---
            # Tiles allocated in the original order; only the dma_start calls
            # for non-critical consts are deferred until after chunk-0's x
            # DMAs, so the first L1 matmuls start ~10 us earlier.
            deferred = []
            A1_s = consts.tile([128, 2048], F16)
            nc.sync.dma_start(out=A1_s, in_=A1_d)
            Se_s = consts.tile([128, 256], FP8)
            deferred.append((Se_s, Se_d))
            So_s = consts.tile([128, 256], FP8)
            deferred.append((So_s, So_d))
            Sa_s = consts.tile([128, 256], FP8)
            deferred.append((Sa_s, Sa_d))
            Sb_s = consts.tile([128, 256], FP8)
            deferred.append((Sb_s, Sb_d))
            W4_s = consts.tile([128, 6 * 128], BF16)
            deferred.append((W4_s, W4_d))
            Wf_s = consts.tile([128, 256], FP8)
            deferred.append((Wf_s, Wf_d))
            t1_s = consts.tile([128, 1], F32)
            nt1_s = consts.tile([128, 1], F32)
            b2_s = consts.tile([128, 1], F32)
            deferred.append((b2_s, b2_d))
            t3_s = consts.tile([128, 1], F32)
            deferred.append((t3_s, t3_d))
            nt3_s = consts.tile([128, 1], F32)
            deferred.append((nt3_s, nt3_d))
            b4_s = consts.tile([128, 1], F32)
            deferred.append((b4_s, b4_d))
            bf_s = consts.tile([10, 1], F32)
            deferred.append((bf_s, bf_d))

            pair2 = dict(two=2)

            def p2(ap):
                return ap.rearrange("p (two n) -> p two n", **pair2)

            # persistent double buffers (halos set once)
            s1_bufs, q_bufs, s3_bufs, s4_bufs = [], [], [], []
            for i in range(2):
                s1b = s1_pool.tile([128, 10 * BH], FP8, name=f"s1b{i}")
                nc.vector.memset(s1b[:, 0:BH], 0.5)          # pad decodes to 0
                nc.vector.memset(s1b[:, 9 * BH:10 * BH], 0.5)
                s1_bufs.append(s1b)
                qb = q_pool.tile([128, 18 * BH], FP8, name=f"qb{i}")
                nc.vector.memset(qb[:, 0:BH], 0.0)
                nc.vector.memset(qb[:, 17 * BH:18 * BH], 0.0)
                q_bufs.append(qb)
                s3_bufs.append(s3_pool.tile([128, 16 * BH], BF16, name=f"s3b{i}"))
                s4_bufs.append(s4_pool.tile([128, 1024], FP8, name=f"s4b{i}"))

            def issue_x(c):
                tiles = []
                for pp in range(4):
                    off = pp * B_CORE * 6 + c * BH
                    xh = xin_pool.tile([128, BH], F16, tag="xh",
                                       name=f"xh_{c}_{pp}")
                    nc.sync.dma_start(out=xh, in_=xh_d[:, off:off + BH])
                    xl = xin_pool.tile([128, BH], F16, tag="xl",
                                       name=f"xl_{c}_{pp}")
                    nc.sync.dma_start(out=xl, in_=xl_d[:, off:off + BH])
                    tiles.append((xh, xl))
                return tiles

            def emit_l4(c):
                # L4: plain bf16, contract (ci,h) over u-parity halves
                s3c = s3_bufs[c % 2]
                s4c = s4_bufs[c % 2]
                s3v = s3c.rearrange("p (u h b) -> p u h b", h=6, b=NB)
                ps4 = psM_pool.tile([128, 1024], F32, tag="psM")
                for h in range(6):
                    for par in range(2):
                        nc.tensor.matmul(
                            ps4[:, 512 * par:512 * (par + 1)],
                            W4_s[:, 128 * h:128 * (h + 1)],
                            s3v[:, par:16:2, h, :],
                            start=(h == 0), stop=(h == 5))
                nc.scalar.activation(s4c, p2(ps4), AF.Sign, bias=b4_s)

            def emit_fc(c):
                # fc: DoubleRow pairs (w=2k, w=2k+1); s4 evicted a full
                # pipeline phase ago, so these matmuls never wait
                s4c = s4_bufs[c % 2]
                psf = psM_pool.tile([10, 64], F32, tag="psM")
                s4v = p2(s4c)
                for k in range(8):
                    nc.tensor.matmul(
                        psf,
                        p2(Wf_s[:, 32 * k:32 * k + 32])[:, :, 0:10],
                        s4v[:, :, 64 * k:64 * (k + 1)],
                        start=(k == 0), stop=(k == 7), perf_mode=DR)
                outc = oc_pool.tile([10, NB], F32)
                nc.vector.tensor_scalar_add(outc, psf, bf_s)
                nc.gpsimd.dma_start(
                    out=y_d[c * NB:(c + 1) * NB, :].rearrange("b j -> j b"),
                    in_=outc)

            x_cur = issue_x(0)
            nc.sync.dma_start(out=t1_s, in_=t1_d)
            nc.sync.dma_start(out=nt1_s, in_=nt1_d)
            for t, d in deferred:
                nc.sync.dma_start(out=t, in_=d)

            def pool_evict(psA, psB, dest, t_s, nt_s, on_act):
                # dest <- OR([psA>=t],[psB>=t]) in {0,1} fp8 (wide pair)
                fa = f_pool.tile([128, 2 * BH], BF16, tag="fa")
                if on_act:
                    nc.scalar.activation(fa, p2(psA)[:, :, 0:BH],
                                         AF.Sign, bias=nt_s)
                else:
                    nc.vector.tensor_scalar(
                        out=fa, in0=p2(psA)[:, :, 0:BH],
                        scalar1=t_s, scalar2=None, op0=ALU.is_ge)
                nc.vector.scalar_tensor_tensor(
                    dest, p2(psB)[:, :, 0:BH], t_s, fa,
                    op0=ALU.is_ge, op1=ALU.max)

            def emit_l1(c, xtiles):
                s1t = s1_bufs[c % 2]
                for g in range(4):                # pair g: strips r=0,1
                    xh, xl = xtiles[g]
                    psE = psM_pool.tile([128, 1024], F32, tag="psM",
                                        name=f"psE_{c}_{g}")
                    psO = psM_pool.tile([128, 1024], F32, tag="psM",
                                        name=f"psO_{c}_{g}")
                    for half, ps in ((0, psE), (1, psO)):
                        for r in range(2):        # two strips run concurrent
                            a = A1_s[64 * r:64 * r + 64,
                                     g * 512 + r * 256 + half * 128:
                                     g * 512 + r * 256 + half * 128 + 128]
                            nc.tensor.matmul(
                                ps[:, 512 * r:512 * r + BH], a,
                                xh[64 * r:64 * r + 64, :],
                                start=True, stop=False,
                                tile_position=(64 * r, 0))
                        for r in range(2):
                            a = A1_s[64 * r:64 * r + 64,
                                     g * 512 + r * 256 + half * 128:
                                     g * 512 + r * 256 + half * 128 + 128]
                            nc.tensor.matmul(
                                ps[:, 512 * r:512 * r + BH], a,
                                xl[64 * r:64 * r + 64, :],
                                start=False, stop=True,
                                tile_position=(64 * r, 0))
                    pool_evict(psE, psO,
                               s1t[:, (2 * g + 1) * BH:(2 * g + 3) * BH],
                               t1_s, nt1_s, g in L1_ACT)

            def emit_l2(c):
                s1t = s1_bufs[c % 2]
                qt = q_bufs[c % 2]
                for g in range(2):
                    pss = [psM_pool.tile([128, 1024], F32, tag="psM",
                                         name=f"ps2_{c}_{g}_{i}")
                           for i in range(4)]

                    def mm_e(i):                          # even-v MM (Se)
                        u = 4 * g + i
                        nc.tensor.matmul(
                            pss[i][:, 0:BH], p2(Se_s),
                            p2(s1t[:, u * BH:(u + 2) * BH]),
                            start=True, stop=True, perf_mode=DR)

                    def mm_o(i):                          # odd-v MM (So)
                        u = 4 * g + i
                        nc.tensor.matmul(
                            pss[i][:, 512:512 + BH], p2(So_s),
                            p2(s1t[:, (u + 1) * BH:(u + 3) * BH]),
                            start=True, stop=True, perf_mode=DR)

                    first, second = (mm_e, mm_o) if g % 2 == 0 else (mm_o, mm_e)
                    for i in range(4):
                        first(i)
                    for i in range(4):
                        second(i)
                    for i, ps in enumerate(pss):          # q tiles v=2u, 2u+1
                        u = 4 * g + i
                        nc.scalar.activation(
                            qt[:, (2 * u + 1) * BH:(2 * u + 3) * BH],
                            p2(ps)[:, :, 0:BH], AF.Sign, bias=b2_s)

            def emit_l3(c):
                qt = q_bufs[c % 2]
                s3t = s3_bufs[c % 2]
                for g in range(8):
                    psA = psM_pool.tile([128, 1024], F32, tag="psM",
                                        name=f"psA_{c}_{g}")
                    psB = psM_pool.tile([128, 1024], F32, tag="psM",
                                        name=f"psB_{c}_{g}")

                    def mm_a(i):                  # pos-left MM (Sa)
                        u3 = 2 * g + i
                        nc.tensor.matmul(
                            psA[:, 512 * i:512 * i + BH], p2(Sa_s),
                            p2(qt[:, u3 * BH:(u3 + 2) * BH]),
                            start=True, stop=True, perf_mode=DR)

                    def mm_b(i):                  # pos-right MM (Sb)
                        u3 = 2 * g + i
                        nc.tensor.matmul(
                            psB[:, 512 * i:512 * i + BH], p2(Sb_s),
                            p2(qt[:, (u3 + 1) * BH:(u3 + 3) * BH]),
                            start=True, stop=True, perf_mode=DR)

                    # alternate stationary order per group: runs of 4 MMs
                    # share one LDWEIGHTS across the group boundary
                    first, second = (mm_a, mm_b) if g % 2 == 0 else (mm_b, mm_a)
                    first(0), first(1), second(0), second(1)
                    # evict the earlier-finished side first (OR is symmetric)
                    pa, pb = (psA, psB) if g % 2 == 0 else (psB, psA)
                    pool_evict(pa, pb,
                               s3t[:, (2 * g) * BH:(2 * g + 2) * BH],
                               t3_s, nt3_s, g in L3_ACT)

            # 2-deep software pipeline: iteration it emits
            #   L3(it-1) | L1(it) | L2(it) | L4+fc(it-2)
            # (L3 first: its q inputs resolved a full iteration ago, so the
            # PE starts immediately instead of waiting on the psum rotation)
            for it in range(N_CHUNKS + 1):
                if 1 <= it <= N_CHUNKS:
                    emit_l3(it - 1)
                if it >= 3:
                    emit_fc(it - 3)
                if it < N_CHUNKS:
                    xtiles = x_cur
                    if it + 1 < N_CHUNKS:
                        x_cur = issue_x(it + 1)
                    emit_l1(it, xtiles)
                    emit_l2(it)
                if it >= 2:
                    emit_l4(it - 2)
            # compressed tail: L4(15) right away, then the last two fc's
            # (fc(13) was already emitted at it=16)
            emit_l4(N_CHUNKS - 1)
            emit_fc(N_CHUNKS - 2)
            emit_fc(N_CHUNKS - 1)


    nc.compile()
    return nc


_PROGRAM = None


def _get_program():
    global _PROGRAM
    if _PROGRAM is None:
        _PROGRAM = build_program()
    return _PROGRAM


def run(trace=False, **inputs):
    inputs = {k: np.asarray(v) for k, v in inputs.items()}
    consts = prepare_host_tensors(
        **{k: inputs[k] for k in
           ("w1", "b1", "w2", "b2", "w3", "b3", "w4", "b4",
            "g1", "be1", "m1", "v1", "g2", "be2", "m2", "v2",
            "g3", "be3", "m3", "v3", "g4", "be4", "m4", "v4", "wf", "bf")})
    x = inputs["x"].astype(np.float32)           # [8192, 1, 6, 128]
    nc = _get_program()
    in_maps = []
    for k in range(N_CORES):
        xc = x[k * B_CORE:(k + 1) * B_CORE, 0]               # [1024, 6, 128]
        # chunk-column order (c, h, b) so L4's rhs slices are contiguous
        xT = np.ascontiguousarray(
            xc.reshape(N_CHUNKS, NB, 6, 128).transpose(0, 2, 1, 3)
            .reshape(B_CORE * 6, 128).T)
        # row-remapped pair layouts: pair p strip r <- x rows 16*(2p+r)-4 ..
        xq = np.zeros((128, 4 * B_CORE * 6), np.float32)
        for pp in range(4):
            for r in range(2):
                base = 16 * (2 * pp + r) - 4
                lo, hi = max(0, -base), min(64, 128 - base)
                xq[64 * r + lo:64 * r + hi,
                   pp * B_CORE * 6:(pp + 1) * B_CORE * 6] = \
                    xT[base + lo:base + hi, :]
        xh = xq.astype(np.float16)
        xl = (xq - xh.astype(np.float32)).astype(np.float16)
        m = {"xh": xh, "xl": xl}
        m.update(consts)
        in_maps.append(m)
    res = run_bass_kernel_spmd(nc, in_maps, list(range(N_CORES)), trace=trace)
    y = np.concatenate([r["y"] for r in res.results], axis=0)
    return y.astype(np.float32), res


def kernel(**inputs):
    y, _ = run(trace=False, **inputs)
    return y
